# revision 23
# baseline (speedup 1.0000x reference)
"""Multi-head attention block (B=16, N=577, C=1024, H=16) on 8 Trainium2 NeuronCores.

Sharding: data-parallel over batch — 2 batch elements per core, no collectives.

Device dataflow per batch element (fully "transposed" so no on-device transposes):
  inputs staged host-side: xT = x^T  [C,N] bf16, wqkvT = w_qkv^T [C,3C] bf16,
  wprojT = w_proj^T [C,C] bf16.
  qT,kT [o,n] <- (wqkvT tile).T @ xT      (o on partitions: per-head [64, n])
  V     [n,o] <- (xT tile).T @ wqkvT      (n on partitions: per-head [m, 64])
  S^T   [m,n] <- (kT_h tile [d,m]).T @ qT_h [d,n]          (d=64 contraction)
  P^T = exp(0.125 * S^T)                  (softmax numerator; max-subtraction skipped:
                                           scaled scores are ~N(0,1), |s|<~10, exp safe)
  OT'[0:64,n] = sum_m V_h[m,d] P^T[m,n];  OT'[64,n] = sum_m P^T[m,n]
      (one matmul: lhsT = [V_h | ones] [m, 65] — sumexp comes free as row 64)
  OT = OT'[0:64] * (1/OT'[64])            (softmax denominator)
  y[n,o] = (OT tile [c,n]).T @ wprojT + b_proj

Schedule: one merged software pipeline. A short prefix computes q/k tiles for
head-pair 0 plus the first half of V; then 16 attention head-pairs (8 per batch)
run back to back.  All remaining QKV / V / projection matmuls are emitted as
~0.5-1.7us "filler" units paced between the S-matmul quads of each pair, so the
PE never waits on the ACT engine's exp chain (which frees the S psum tiles).
S-matmul quads are emitted paired (head A rows 0:64, head B rows 64:128) so the
PE runs both heads' score matmuls concurrently in disjoint row-groups.
"""

import os
import sys

import numpy as np

if "/opt/trn_rl_repo" not in sys.path:
    sys.path.insert(0, "/opt/trn_rl_repo")

import ml_dtypes

B, N, C = 16, 577, 1024
H, D = 16, 64
P = 128
CT = C // P  # 8 contraction tiles
NT = 5  # n(row) tiles of 128: 4*128 + 65
NTS = [128, 128, 128, 128, 65]
NCH = [(0, 512), (512, 65)]  # free-dim chunks of 577 (psum bank = 512 fp32)
NCORES = 8
BPC = B // NCORES  # batches per core

_CACHE = {}
LAST_RESULT = None


def _ensure_ntff_hook():
    """Install antenv.axon_hooks with a ctypes-based NTFF profile hook if the
    environment's antenv package lacks it (mirrors trn_boot._ntff_profile_via_ctypes).
    Without this, run_bass_kernel_spmd(trace=True) silently skips tracing."""
    try:
        from antenv import axon_hooks  # noqa: F401

        return
    except ImportError:
        pass
    import contextlib
    import ctypes
    import types

    import antenv

    so_path = "/opt/axon/libaxon_pjrt.so"
    mod = types.ModuleType("antenv.axon_hooks")
    _state = {"hook": None, "set": False}

    def _make_hook():
        if not os.path.exists(so_path):
            return None
        lib = ctypes.CDLL(so_path)
        if not hasattr(lib, "axon_start_nrt_profile"):
            return None
        lib.axon_start_nrt_profile.argtypes = [
            ctypes.POINTER(ctypes.c_int64),
            ctypes.c_size_t,
        ]
        lib.axon_start_nrt_profile.restype = ctypes.c_int64
        lib.axon_stop_nrt_profile.argtypes = [ctypes.c_char_p]
        lib.axon_stop_nrt_profile.restype = ctypes.c_int64

        @contextlib.contextmanager
        def _hook(output_dir, device_ids):
            import jax

            jax.devices()
            if device_ids:
                ids = (ctypes.c_int64 * len(device_ids))(*device_ids)
                rc = lib.axon_start_nrt_profile(ids, len(device_ids))
            else:
                rc = lib.axon_start_nrt_profile(None, 0)
            if rc != 0:
                raise RuntimeError(f"axon_start_nrt_profile rc={rc}")
            try:
                yield
            finally:
                n = lib.axon_stop_nrt_profile(str(output_dir).encode())
                print(f"ntff profile: {n} file(s) written to {output_dir}", file=sys.stderr)

        return _hook

    def set_axon_ntff_profile_hook(h):
        _state["hook"] = h
        _state["set"] = True

    def get_axon_ntff_profile_hook():
        if not _state["set"]:
            set_axon_ntff_profile_hook(_make_hook())
        return _state["hook"]

    mod.set_axon_ntff_profile_hook = set_axon_ntff_profile_hook
    mod.get_axon_ntff_profile_hook = get_axon_ntff_profile_hook
    sys.modules["antenv.axon_hooks"] = mod
    antenv.axon_hooks = mod


def _build_nc():
    import concourse.bass as bass
    import concourse.tile as tile
    from concourse import bacc, mybir

    dtb = mybir.dt.bfloat16
    dtf = mybir.dt.float32
    Exp = mybir.ActivationFunctionType.Exp

    nc = bacc.Bacc(None, target_bir_lowering=False)

    # wq/wp are host-side pre-split to [P, CT, cols] and wq's columns are
    # permuted so first-needed blocks are contiguous: [ot0 q | ot8 k | V oc0 |
    # q ot1-7 | k ot9-15 | V oc1].  This lets the whole weight load run as a
    # handful of large, wide-line DMAs (each DMA has a ~0.6us queue cost).
    xt = nc.dram_tensor("xt", [BPC, C, N], dtb, kind="ExternalInput")
    wq = nc.dram_tensor("wqkvT", [P, CT, 3 * C], dtb, kind="ExternalInput")
    wp = nc.dram_tensor("wprojT", [P, CT, C], dtb, kind="ExternalInput")
    bqk = nc.dram_tensor("bqk", [P, 16], dtf, kind="ExternalInput")
    bv = nc.dram_tensor("bv", [C], dtb, kind="ExternalInput")
    bpr = nc.dram_tensor("bproj", [C], dtb, kind="ExternalInput")
    y = nc.dram_tensor("y", [BPC, N, C], dtb, kind="ExternalOutput")

    from contextlib import ExitStack

    with tile.TileContext(nc) as tc:
        with ExitStack() as ctx:
            consts = ctx.enter_context(tc.tile_pool(name="consts", bufs=1))
            wpool = ctx.enter_context(tc.tile_pool(name="weights", bufs=1))
            xpool = ctx.enter_context(tc.tile_pool(name="xin", bufs=2))
            qkpool = ctx.enter_context(tc.tile_pool(name="qk", bufs=2))
            vpool = ctx.enter_context(tc.tile_pool(name="vv", bufs=2))
            epool = ctx.enter_context(tc.tile_pool(name="est", bufs=4))
            opool = ctx.enter_context(tc.tile_pool(name="ot", bufs=2))
            n2 = ctx.enter_context(tc.tile_pool(name="n2", bufs=3))
            n1 = ctx.enter_context(tc.tile_pool(name="n1", bufs=2))
            outpool = ctx.enter_context(tc.tile_pool(name="outs", bufs=2))
            # PSUM budget (8 banks of 2KB): psS 2x[P,640]f32 = 4 banks
            # (shared by S tiles and PV outputs -- PV allocations land on
            # buffers freed by the mt4 exps they already depend on), psB
            # 4x[P,512] = 4 banks for the filler accumulators.
            psS = ctx.enter_context(tc.tile_pool(name="psS", bufs=2, space="PSUM"))
            psB = ctx.enter_context(tc.tile_pool(name="psB", bufs=4, space="PSUM"))

            wq_sb = wpool.tile([P, CT, 3 * C], dtb, tag="wq")
            wp_sb = wpool.tile([P, CT, C], dtb, tag="wp")
            bqk_sb = consts.tile([P, 16], dtf, tag="bqk")
            bvb_sb = consts.tile([P, C], dtb, tag="bvb")
            bpb_sb = consts.tile([P, C], dtb, tag="bpb")

            # ---------------- DMA emission (3 queues, few large DMAs) -------
            # permuted wq column offsets (see dram tensor comment)
            def qcol(ot):
                if ot == 0:
                    return 0
                if ot == 8:
                    return 128
                if 1 <= ot <= 7:
                    return 768 + (ot - 1) * P
                return 1664 + (ot - 9) * P

            def vcol(oc):
                return 256 if oc == 0 else 2560

            def dma_x(b, x_sb):
                xb = xt[b].rearrange("(ct p) n -> p ct n", p=P)
                nc.gpsimd.dma_start(out=x_sb[:, 0:4], in_=xb[:, 0:4])
                nc.sync.dma_start(out=x_sb[:, 4:8], in_=xb[:, 4:8])

            x0 = xpool.tile([P, CT, N], dtb, tag="x")
            x1 = xpool.tile([P, CT, N], dtb, tag="x")

            # Strict priority order on the two fast rings (sync/gpsimd); the
            # scalar ring only carries tiny bias loads (it is slow, and its
            # triggers must clear before the first exps).  First-needed bytes
            # (x0 + ot0/ot8 + V oc0 weights) are balanced across both rings.
            nc.scalar.dma_start(out=bqk_sb[:], in_=bqk[:])
            nc.sync.dma_start(out=wq_sb[:, :, 0:256], in_=wq[:, :, 0:256])
            dma_x(0, x0)                      # gpsimd: ct0-3, sync: ct4-7
            # V oc0 cols split across both fast rings right after x0
            nc.sync.dma_start(out=wq_sb[:, :, 256:512], in_=wq[:, :, 256:512])
            nc.gpsimd.dma_start(out=wq_sb[:, :, 512:768], in_=wq[:, :, 512:768])
            nc.scalar.dma_start(
                out=bvb_sb[:], in_=bass.AP(tensor=bv, offset=0, ap=[[0, P], [1, C]])
            )
            nc.scalar.dma_start(
                out=bpb_sb[:], in_=bass.AP(tensor=bpr, offset=0, ap=[[0, P], [1, C]])
            )
            # Remaining weights, split by ct-halves across the two big queues.
            nc.sync.dma_start(out=wq_sb[:, 0:4, 768:3072], in_=wq[:, 0:4, 768:3072])
            nc.gpsimd.dma_start(out=wq_sb[:, 4:8, 768:3072], in_=wq[:, 4:8, 768:3072])
            dma_x(1, x1)
            nc.sync.dma_start(out=wp_sb[:, 0:4], in_=wp[:, 0:4])
            nc.gpsimd.dma_start(out=wp_sb[:, 4:8], in_=wp[:, 4:8])

            # ---------------- emit helpers ----------------
            def emit_qk_half(x_sb, qk_sb, ot, half, cell):
                """half 0: alloc psB + MMs ct0-3 (F=512); half 1: MMs ct4-7 +
                bias evac; then 8 MMs F=65 into a 2nd psB + evac."""
                wc = qcol(ot)
                if half == 0:
                    cell["ps"] = psB.tile([P, 512], dtf, tag="psB", name="psb_t")
                    for ct in range(4):
                        nc.tensor.matmul(
                            cell["ps"][:],
                            lhsT=wq_sb[:, ct, wc:wc + P],
                            rhs=x_sb[:, ct, 0:512],
                            start=(ct == 0),
                            stop=False,
                        )
                    return
                ps = cell.pop("ps")
                for ct in range(4, CT):
                    nc.tensor.matmul(
                        ps[:],
                        lhsT=wq_sb[:, ct, wc:wc + P],
                        rhs=x_sb[:, ct, 0:512],
                        start=False,
                        stop=(ct == CT - 1),
                    )
                nc.vector.tensor_scalar_add(
                    out=qk_sb[:, ot, 0:512], in0=ps[:], scalar1=bqk_sb[:, ot:ot + 1]
                )
                ps2 = psB.tile([P, 512], dtf, tag="psB")
                for ct in range(CT):
                    nc.tensor.matmul(
                        ps2[:, 0:65],
                        lhsT=wq_sb[:, ct, wc:wc + P],
                        rhs=x_sb[:, ct, 512:577],
                        start=(ct == 0),
                        stop=(ct == CT - 1),
                    )
                nc.vector.tensor_scalar_add(
                    out=qk_sb[:, ot, 512:577],
                    in0=ps2[:, 0:65],
                    scalar1=bqk_sb[:, ot:ot + 1],
                )

            def emit_v_half(x_sb, v_sb, nt, oc, half, cell):
                nh = NTS[nt]
                wc = vcol(oc)
                if half == 0:
                    cell["ps"] = psB.tile([P, 512], dtf, tag="psB", name="psb_t")
                    for ct in range(4):
                        nc.tensor.matmul(
                            cell["ps"][:nh],
                            lhsT=x_sb[:, ct, nt * P:nt * P + nh],
                            rhs=wq_sb[:, ct, wc:wc + 512],
                            start=(ct == 0),
                            stop=False,
                        )
                    return
                ps = cell.pop("ps")
                for ct in range(4, CT):
                    nc.tensor.matmul(
                        ps[:nh],
                        lhsT=x_sb[:, ct, nt * P:nt * P + nh],
                        rhs=wq_sb[:, ct, wc:wc + 512],
                        start=False,
                        stop=(ct == CT - 1),
                    )
                # single strided evac: 8 head-slices + bias in one DVE op
                vv = v_sb[:nh, nt, oc * 8 * 65:(oc + 1) * 8 * 65].rearrange(
                    "p (h c) -> p h c", c=65
                )
                nc.vector.tensor_add(
                    out=vv[:, :, 0:64],
                    in0=ps[:nh].rearrange("p (h c) -> p h c", c=64),
                    in1=bvb_sb[:nh, oc * 512:(oc + 1) * 512].rearrange(
                        "p (h c) -> p h c", c=64
                    ),
                )

            def alloc_v():
                v_sb = vpool.tile([P, NT, H * 65], dtb, tag="v")
                v4 = v_sb[:].rearrange("p nt (h c) -> p nt h c", c=65)
                nc.vector.memset(v4[:, :, :, 64], 1.0)
                return v_sb

            def emit_proj_half(ot_sb, b, nt, oc, half, cell):
                nh = NTS[nt]
                if half == 0:
                    cell["ps"] = psB.tile([P, 512], dtf, tag="psB", name="psb_t")
                    for ct in range(4):
                        nc.tensor.matmul(
                            cell["ps"][:nh],
                            lhsT=ot_sb[:, ct, nt * P:nt * P + nh],
                            rhs=wp_sb[:, ct, oc * 512:(oc + 1) * 512],
                            start=(ct == 0),
                            stop=False,
                        )
                    return
                ps = cell.pop("ps")
                for ct in range(4, CT):
                    nc.tensor.matmul(
                        ps[:nh],
                        lhsT=ot_sb[:, ct, nt * P:nt * P + nh],
                        rhs=wp_sb[:, ct, oc * 512:(oc + 1) * 512],
                        start=False,
                        stop=(ct == CT - 1),
                    )
                outt = outpool.tile([P, 512], dtb, tag="out")
                nc.vector.tensor_add(
                    out=outt[:nh],
                    in0=ps[:nh],
                    in1=bpb_sb[:nh, oc * 512:(oc + 1) * 512],
                )
                eng = nc.sync if (nt + oc) % 2 == 0 else nc.gpsimd
                eng.dma_start(
                    out=y[b, nt * P:nt * P + nh, oc * 512:(oc + 1) * 512],
                    in_=outt[:nh],
                )

            def s_quad(qk_sb, hp, mt, estA, estB):
                """Score matmuls for head pair hp, row-tile mt, emitted paired
                (head A rows 0:64 / head B rows 64:128 run concurrently in
                disjoint PE row groups), then the two exps on ACT."""
                ob = hp
                mh = NTS[mt]
                psa = psS.tile([P, 640], dtf, tag="psS")
                psb = psS.tile([P, 640], dtf, tag="psS")
                for (c0, cw) in NCH:
                    nc.tensor.matmul(
                        psa[:mh, c0:c0 + cw],
                        lhsT=qk_sb[0:64, 8 + ob, mt * P:mt * P + mh],
                        rhs=qk_sb[0:64, ob, c0:c0 + cw],
                    )
                    nc.tensor.matmul(
                        psb[:mh, c0:c0 + cw],
                        lhsT=qk_sb[64:128, 8 + ob, mt * P:mt * P + mh],
                        rhs=qk_sb[64:128, ob, c0:c0 + cw],
                    )
                nc.scalar.activation(
                    out=estA[:mh, mt, :], in_=psa[:mh, :N], func=Exp, scale=0.125
                )
                nc.scalar.activation(
                    out=estB[:mh, mt, :], in_=psb[:mh, :N], func=Exp, scale=0.125
                )

            def pv_head(v_sb, est, h):
                pso = psS.tile([P, 640], dtf, tag="psS")
                for (c0, cw) in NCH:
                    for mt in range(NT):
                        mh = NTS[mt]
                        nc.tensor.matmul(
                            pso[:65, c0:c0 + cw],
                            lhsT=v_sb[:mh, mt, h * 65:h * 65 + 65],
                            rhs=est[:mh, mt, c0:c0 + cw],
                            start=(mt == 0),
                            stop=(mt == NT - 1),
                        )
                return pso

            def evac_head(pso, ot_sb, ob, p0):
                """PSUM evac + softmax denominator; returns deferred mul.
                Single DVE copy frees the pso banks in one shot (no ACT work,
                keeping the exp chain unobstructed)."""
                o65 = n2.tile([65, N], dtf, tag="o65", bufs=2)
                nc.vector.tensor_copy(out=o65[:], in_=pso[:65, :N])
                s1 = n1.tile([1, N], dtf, tag="s1")
                nc.vector.tensor_copy(out=s1[0:1, :], in_=o65[64:65, :])
                rec = n1.tile([1, N], dtf, tag="rec")
                nc.vector.reciprocal_approx_fast(out=rec[0:1, :], in_=s1[0:1, :])
                recb = n2.tile([64, N], dtf, tag="recb", bufs=2)
                nc.gpsimd.partition_broadcast(recb[:], rec[0:1, :])

                def mk():
                    nc.vector.tensor_mul(
                        out=ot_sb[p0:p0 + 64, ob, :], in0=o65[0:64, :], in1=recb[:]
                    )

                return mk

            # ---------------- filler unit machinery ----------------
            qk0 = qkpool.tile([P, 16, N], dtb, tag="qk")
            qk1 = qkpool.tile([P, 16, N], dtb, tag="qk")
            v0 = alloc_v()
            v1 = alloc_v()
            ot0 = opool.tile([P, CT, N], dtb, tag="ot")
            ot1 = opool.tile([P, CT, N], dtb, tag="ot")
            xs = [x0, x1]
            qks = [qk0, qk1]
            vs = [v0, v1]
            ots = [ot0, ot1]

            units = []  # (cost_us, req_pair, fn)

            def add_qk_units(b, ot, req):
                cell = {}
                units.append(
                    (0.9, req, lambda b=b, ot=ot, cell=cell: emit_qk_half(
                        xs[b], qks[b], ot, 0, cell))
                )
                units.append(
                    (1.1, req, lambda b=b, ot=ot, cell=cell: emit_qk_half(
                        xs[b], qks[b], ot, 1, cell))
                )

            def add_v_units(b, nt, oc, req):
                cell = {}
                units.append(
                    (0.9, req, lambda b=b, nt=nt, oc=oc, cell=cell: emit_v_half(
                        xs[b], vs[b], nt, oc, 0, cell))
                )
                units.append(
                    (1.0, req, lambda b=b, nt=nt, oc=oc, cell=cell: emit_v_half(
                        xs[b], vs[b], nt, oc, 1, cell))
                )

            def add_proj_units(b, nt, oc, req):
                cell = {}
                units.append(
                    (0.9, req, lambda b=b, nt=nt, oc=oc, cell=cell: emit_proj_half(
                        ots[b], b, nt, oc, 0, cell))
                )
                units.append(
                    (1.0, req, lambda b=b, nt=nt, oc=oc, cell=cell: emit_proj_half(
                        ots[b], b, nt, oc, 1, cell))
                )

            # filler list in rough need order (req = global pair index that
            # needs the unit's output; 99 = not needed by any pair)
            for t in range(1, 8):
                add_qk_units(0, t, t)        # q tile for b0 pair t
                add_qk_units(0, 8 + t, t)    # k tile for b0 pair t
            for nt in range(NT):
                add_v_units(0, nt, 1, 4)     # V0 oc1: b0 pairs 4-7
            add_qk_units(1, 0, 8)
            add_qk_units(1, 8, 8)
            for nt in range(NT):
                add_v_units(1, nt, 0, 8)     # V1 oc0: b1 pairs 8-11
            for t in range(1, 8):
                add_qk_units(1, t, 8 + t)
                add_qk_units(1, 8 + t, 8 + t)
            for nt in range(NT):
                add_v_units(1, nt, 1, 12)    # V1 oc1: b1 pairs 12-15
            for nt in range(NT):
                for oc in range(2):
                    add_proj_units(0, nt, oc, 99)
            # two proj(b1) ct0-3 halves as the very last fillers (ready after
            # pair 11; at most 2 psB tiles may stay un-evacuated until the
            # tail, matching psB bufs=2)
            proj1_chunks = [(nt, oc) for nt in range(NT) for oc in range(2)]
            proj1_cells = {ch: {} for ch in proj1_chunks}
            for ch in proj1_chunks[:4]:
                units.append(
                    (0.9, 99, lambda ch=ch: emit_proj_half(
                        ot1, 1, ch[0], ch[1], 0, proj1_cells[ch]))
                )

            total_cost = sum(u[0] for u in units)
            state = {"i_next": 0, "spent": 0.0}

            def force_req(k):
                kept = []
                for u in units[state["i_next"]:]:
                    if u[1] <= k:
                        u[2]()
                        state["spent"] += u[0]
                    else:
                        kept.append(u)
                units[state["i_next"]:] = kept

            def drain(budget):
                # emit units while at least half the next unit fits the budget
                while state["i_next"] < len(units):
                    u = units[state["i_next"]]
                    if budget < 0.5 * u[0]:
                        break
                    state["i_next"] += 1
                    u[2]()
                    state["spent"] += u[0]
                    budget -= u[0]

            # ---------------- prefix: pair-0 deps, PE-dense ----------------
            # Order follows DMA arrival: ot0 cols land first (scalar queue),
            # V oc0 cols next (sync), ot8 cols last (scalar).
            cell = {}
            emit_qk_half(x0, qk0, 0, 0, cell)
            emit_qk_half(x0, qk0, 0, 1, cell)
            for nt in range(NT):
                cell = {}
                emit_v_half(x0, v0, nt, 0, 0, cell)
                emit_v_half(x0, v0, nt, 0, 1, cell)
            cell = {}
            emit_qk_half(x0, qk0, 8, 0, cell)
            emit_qk_half(x0, qk0, 8, 1, cell)

            # ---------------- 16 head pairs ----------------
            for k in range(16):
                b, hp = k // 8, k % 8
                force_req(k)
                estA = epool.tile([P, NT, N], dtb, tag="est")
                estB = epool.tile([P, NT, N], dtb, tag="est")
                for mt in range(NT):
                    s_quad(qks[b], hp, mt, estA, estB)
                    drain(0.95)
                psoA = pv_head(vs[b], estA, 2 * hp)
                mulA = evac_head(psoA, ots[b], hp, 0)
                drain(0.9)
                psoB = pv_head(vs[b], estB, 2 * hp + 1)
                mulB = evac_head(psoB, ots[b], hp, 64)
                drain(0.9)
                mulA()
                mulB()
                # keep global filler consumption on pace, holding back a
                # reserve so pair 15's quads still have inter-quad fillers
                target = (k + 1) * (total_cost - 5.0) / 16.0
                drain(target - state["spent"])

            # ---------------- tail: proj(b1), software-pipelined ----------
            # h0 (ct0-3, ready after pair 11) of chunk i+1 runs before h1
            # (ct4-7, gated on pair-15 muls) of chunk i, so the PE stays busy
            # across the last-pair normalize latency.  Chunks whose h0 was
            # already emitted as a filler (cell non-empty) are skipped.
            pend = []
            for ch in proj1_chunks:
                c = proj1_cells[ch]
                if "ps" not in c:
                    emit_proj_half(ot1, 1, ch[0], ch[1], 0, c)
                pend.append((ch, c))
                while len(pend) >= 4:
                    ch0, c0 = pend.pop(0)
                    emit_proj_half(ot1, 1, ch0[0], ch0[1], 1, c0)
            for ch0, c0 in pend:
                emit_proj_half(ot1, 1, ch0[0], ch0[1], 1, c0)
    nc.compile()
    return nc


def kernel(x, w_qkv, b_qkv, w_proj, b_proj):
    global LAST_RESULT
    _ensure_ntff_hook()
    from concourse.bass_utils import run_bass_kernel_spmd

    bf16 = ml_dtypes.bfloat16
    x = np.asarray(x, dtype=np.float32)
    w_qkv = np.asarray(w_qkv, dtype=np.float32)
    b_qkv = np.asarray(b_qkv, dtype=np.float32)
    w_proj = np.asarray(w_proj, dtype=np.float32)
    b_proj = np.asarray(b_proj, dtype=np.float32)

    xT = np.ascontiguousarray(np.transpose(x, (0, 2, 1))).astype(bf16)  # [B, C, N]
    wqkvT = w_qkv.T  # [C, 3C]
    # permute columns first-needed-first, then split rows into [P, CT, 3C]
    wq_perm = np.concatenate(
        [
            wqkvT[:, 0:128],       # ot0 (q head pair 0)
            wqkvT[:, 1024:1152],   # ot8 (k head pair 0)
            wqkvT[:, 2048:2560],   # V oc0
            wqkvT[:, 128:1024],    # q ot1-7
            wqkvT[:, 1152:2048],   # k ot9-15
            wqkvT[:, 2560:3072],   # V oc1
        ],
        axis=1,
    )
    wqh = np.ascontiguousarray(
        wq_perm.reshape(CT, P, 3 * C).transpose(1, 0, 2)
    ).astype(bf16)  # [P, CT, 3C]
    wph = np.ascontiguousarray(
        w_proj.T.reshape(CT, P, C).transpose(1, 0, 2)
    ).astype(bf16)  # [P, CT, C]
    bqk = np.ascontiguousarray(b_qkv[:2 * C].reshape(16, P).T).astype(np.float32)
    bv = np.ascontiguousarray(b_qkv[2 * C:]).astype(bf16)
    bpr = np.ascontiguousarray(b_proj).astype(bf16)

    in_maps = []
    for i in range(NCORES):
        in_maps.append(
            {
                "xt": np.ascontiguousarray(xT[i * BPC:(i + 1) * BPC]),
                "wqkvT": wqh,
                "wprojT": wph,
                "bqk": bqk,
                "bv": bv,
                "bproj": bpr,
            }
        )

    if "nc" not in _CACHE:
        _CACHE["nc"] = _build_nc()
    nc = _CACHE["nc"]

    res = run_bass_kernel_spmd(nc, in_maps, core_ids=list(range(NCORES)))
    LAST_RESULT = res
    out = np.concatenate([r["y"] for r in res.results], axis=0)
    return np.ascontiguousarray(out.astype(np.float32))


if __name__ == "__main__":
    rng = np.random.default_rng(0)
    x = rng.standard_normal((B, N, C), dtype=np.float32)
    w_qkv = rng.standard_normal((3 * C, C), dtype=np.float32) * C ** -0.5
    b_qkv = rng.standard_normal(3 * C).astype(np.float32) * 0.02
    w_proj = rng.standard_normal((C, C), dtype=np.float32) * C ** -0.5
    b_proj = rng.standard_normal(C).astype(np.float32) * 0.02
    out = kernel(x=x, w_qkv=w_qkv, b_qkv=b_qkv, w_proj=w_proj, b_proj=b_proj)
    print(out.shape, out.dtype)


# revision 24
# speedup vs baseline: 1.1057x; 1.1057x over previous
"""Multi-head attention block (B=16, N=577, C=1024, H=16) on 8 Trainium2 NeuronCores.

Sharding: data-parallel over batch — 2 batch elements per core, no collectives.

Device dataflow per batch element (fully "transposed" so no on-device transposes):
  inputs staged host-side: xT = x^T  [C,N] bf16, wqkvT = w_qkv^T [C,3C] bf16,
  wprojT = w_proj^T [C,C] bf16.
  qT,kT [o,n] <- (wqkvT tile).T @ xT      (o on partitions: per-head [64, n])
  V     [n,o] <- (xT tile).T @ wqkvT      (n on partitions: per-head [m, 64])
  S^T   [m,n] <- (kT_h tile [d,m]).T @ qT_h [d,n]          (d=64 contraction)
  P^T = exp(0.125 * S^T)                  (softmax numerator; max-subtraction skipped:
                                           scaled scores are ~N(0,1), |s|<~10, exp safe)
  OT'[0:64,n] = sum_m V_h[m,d] P^T[m,n];  OT'[64,n] = sum_m P^T[m,n]
      (one matmul: lhsT = [V_h | ones] [m, 65] — sumexp comes free as row 64)
  OT = OT'[0:64] * (1/OT'[64])            (softmax denominator)
  y[n,o] = (OT tile [c,n]).T @ wprojT + b_proj
"""

import os
import sys

import numpy as np

if "/opt/trn_rl_repo" not in sys.path:
    sys.path.insert(0, "/opt/trn_rl_repo")

import ml_dtypes

B, N, C = 16, 577, 1024
H, D = 16, 64
P = 128
CT = C // P  # 8 contraction tiles
NT = 5  # n(row) tiles of 128: 4*128 + 65
NTS = [128, 128, 128, 128, 65]
NCH = [(0, 512), (512, 65)]  # free-dim chunks of 577 (psum bank = 512 fp32)
NCORES = 8
BPC = B // NCORES  # batches per core

_CACHE = {}
LAST_RESULT = None


def _ensure_ntff_hook():
    """Install antenv.axon_hooks with a ctypes-based NTFF profile hook if the
    environment's antenv package lacks it (mirrors trn_boot._ntff_profile_via_ctypes).
    Without this, run_bass_kernel_spmd(trace=True) silently skips tracing."""
    try:
        from antenv import axon_hooks  # noqa: F401

        return
    except ImportError:
        pass
    import contextlib
    import ctypes
    import types

    import antenv

    so_path = "/opt/axon/libaxon_pjrt.so"
    mod = types.ModuleType("antenv.axon_hooks")
    _state = {"hook": None, "set": False}

    def _make_hook():
        if not os.path.exists(so_path):
            return None
        lib = ctypes.CDLL(so_path)
        if not hasattr(lib, "axon_start_nrt_profile"):
            return None
        lib.axon_start_nrt_profile.argtypes = [
            ctypes.POINTER(ctypes.c_int64),
            ctypes.c_size_t,
        ]
        lib.axon_start_nrt_profile.restype = ctypes.c_int64
        lib.axon_stop_nrt_profile.argtypes = [ctypes.c_char_p]
        lib.axon_stop_nrt_profile.restype = ctypes.c_int64

        @contextlib.contextmanager
        def _hook(output_dir, device_ids):
            import jax

            jax.devices()
            if device_ids:
                ids = (ctypes.c_int64 * len(device_ids))(*device_ids)
                rc = lib.axon_start_nrt_profile(ids, len(device_ids))
            else:
                rc = lib.axon_start_nrt_profile(None, 0)
            if rc != 0:
                raise RuntimeError(f"axon_start_nrt_profile rc={rc}")
            try:
                yield
            finally:
                n = lib.axon_stop_nrt_profile(str(output_dir).encode())
                print(f"ntff profile: {n} file(s) written to {output_dir}", file=sys.stderr)

        return _hook

    def set_axon_ntff_profile_hook(h):
        _state["hook"] = h
        _state["set"] = True

    def get_axon_ntff_profile_hook():
        if not _state["set"]:
            set_axon_ntff_profile_hook(_make_hook())
        return _state["hook"]

    mod.set_axon_ntff_profile_hook = set_axon_ntff_profile_hook
    mod.get_axon_ntff_profile_hook = get_axon_ntff_profile_hook
    sys.modules["antenv.axon_hooks"] = mod
    antenv.axon_hooks = mod


def _build_nc():
    import concourse.bass as bass
    import concourse.tile as tile
    from concourse import bacc, mybir

    dtb = mybir.dt.bfloat16
    dtf = mybir.dt.float32
    Exp = mybir.ActivationFunctionType.Exp

    nc = bacc.Bacc(None, target_bir_lowering=False)

    xt = nc.dram_tensor("xt", [BPC, C, N], dtb, kind="ExternalInput")
    wq = nc.dram_tensor("wqkvT", [C, 3 * C], dtb, kind="ExternalInput")
    wp = nc.dram_tensor("wprojT", [C, C], dtb, kind="ExternalInput")
    bqk = nc.dram_tensor("bqk", [P, 16], dtf, kind="ExternalInput")
    bv = nc.dram_tensor("bv", [C], dtb, kind="ExternalInput")
    bpr = nc.dram_tensor("bproj", [C], dtb, kind="ExternalInput")
    y = nc.dram_tensor("y", [BPC, N, C], dtb, kind="ExternalOutput")

    from contextlib import ExitStack

    with tile.TileContext(nc) as tc:
        with ExitStack() as ctx:
            consts = ctx.enter_context(tc.tile_pool(name="consts", bufs=1))
            wpool = ctx.enter_context(tc.tile_pool(name="weights", bufs=1))
            xpool = ctx.enter_context(tc.tile_pool(name="xin", bufs=2))
            qkpool = ctx.enter_context(tc.tile_pool(name="qk", bufs=2))
            vpool = ctx.enter_context(tc.tile_pool(name="vv", bufs=2))
            epool = ctx.enter_context(tc.tile_pool(name="est", bufs=4))
            opool = ctx.enter_context(tc.tile_pool(name="ot", bufs=2))
            n2 = ctx.enter_context(tc.tile_pool(name="n2", bufs=3))
            n1 = ctx.enter_context(tc.tile_pool(name="n1", bufs=2))
            outpool = ctx.enter_context(tc.tile_pool(name="outs", bufs=2))
            psA = ctx.enter_context(tc.tile_pool(name="psA", bufs=3, space="PSUM"))
            psB = ctx.enter_context(tc.tile_pool(name="psB", bufs=2, space="PSUM"))

            wq_sb = wpool.tile([P, CT, 3 * C], dtb, tag="wq")
            wp_sb = wpool.tile([P, CT, C], dtb, tag="wp")
            bqk_sb = consts.tile([P, 16], dtf, tag="bqk")
            ones1 = consts.tile([1, P], dtb, tag="ones1")
            nc.vector.memset(ones1[:], 1.0)
            bvb_sb = consts.tile([P, C], dtb, tag="bvb")
            bpb_sb = consts.tile([P, C], dtb, tag="bpb")

            def load_x(b):
                x_sb = xpool.tile([P, CT, N], dtb, tag="x")
                xb = xt[b].rearrange("(ct p) n -> p ct n", p=P)
                for ct in range(CT):
                    eng = nc.sync if ct % 2 == 0 else nc.gpsimd
                    eng.dma_start(out=x_sb[:, ct], in_=xb[:, ct])
                return x_sb

            def emit_qk_tile(x_sb, qk_sb, ot, wide):
                """qT/kT o-tile: psum [o,n] accumulated over ct, DVE evac+bias.
                wide=True uses one 2-bank psA tile + single evac (phase 1,
                when psA is otherwise idle); wide=False uses two 1-bank psB
                tiles (attention-phase filler)."""
                if wide:
                    ps = psA.tile([P, 640], dtf, tag="psA")
                    for (c0, cw) in NCH:
                        for ct in range(CT):
                            nc.tensor.matmul(
                                ps[:, c0:c0 + cw],
                                lhsT=wq_sb[:, ct, ot * P:(ot + 1) * P],
                                rhs=x_sb[:, ct, c0:c0 + cw],
                                start=(ct == 0),
                                stop=(ct == CT - 1),
                            )
                    nc.vector.tensor_scalar_add(
                        out=qk_sb[:, ot, :],
                        in0=ps[:, :N],
                        scalar1=bqk_sb[:, ot:ot + 1],
                    )
                    return
                for (c0, cw) in NCH:
                    ps = psB.tile([P, 512], dtf, tag="psB")
                    for ct in range(CT):
                        nc.tensor.matmul(
                            ps[:, :cw],
                            lhsT=wq_sb[:, ct, ot * P:(ot + 1) * P],
                            rhs=x_sb[:, ct, c0:c0 + cw],
                            start=(ct == 0),
                            stop=(ct == CT - 1),
                        )
                    nc.vector.tensor_scalar_add(
                        out=qk_sb[:, ot, c0:c0 + cw],
                        in0=ps[:, :cw],
                        scalar1=bqk_sb[:, ot:ot + 1],
                    )

            def emit_v_chunk(x_sb, v_sb, nt, oc):
                """V 512-col chunk: psum [n,o], scatter into per-head 65-slots."""
                nh = NTS[nt]
                ps = psB.tile([P, 512], dtf, tag="psB")
                for ct in range(CT):
                    nc.tensor.matmul(
                        ps[:nh],
                        lhsT=x_sb[:, ct, nt * P:nt * P + nh],
                        rhs=wq_sb[:, ct, 2 * C + oc * 512:2 * C + (oc + 1) * 512],
                        start=(ct == 0),
                        stop=(ct == CT - 1),
                    )
                vv = v_sb[:nh, nt, oc * 8 * 65:(oc + 1) * 8 * 65].rearrange(
                    "p (h c) -> p h c", c=65
                )
                nc.vector.tensor_add(
                    out=vv[:, :, 0:64],
                    in0=ps[:nh].rearrange("p (h c) -> p h c", c=64),
                    in1=bvb_sb[:nh, oc * 512:(oc + 1) * 512].rearrange(
                        "p (h c) -> p h c", c=64
                    ),
                )

            def alloc_v(b):
                v_sb = vpool.tile([P, NT, H * 65], dtb, tag="v")
                v4 = v_sb[:].rearrange("p nt (h c) -> p nt h c", c=65)
                nc.vector.memset(v4[:, :, :, 64], 1.0)
                return v_sb

            def emit_pair(qk_sb, v_sb, ot_sb, hp, act_help=True):
                """Head pair: S^T (row-group interleaved), exp, PV(+sumexp),
                fast OT' evac. Returns deferred normalize-mul thunks."""
                ob = hp
                estA = epool.tile([P, NT, N], dtb, tag="est")
                estB = epool.tile([P, NT, N], dtb, tag="est")
                for mt in range(NT):
                    mh = NTS[mt]
                    psa = psA.tile([P, 640], dtf, tag="psA")
                    psb = psA.tile([P, 640], dtf, tag="psA")
                    for (c0, cw) in NCH:
                        nc.tensor.matmul(
                            psa[:mh, c0:c0 + cw],
                            lhsT=qk_sb[0:64, 8 + ob, mt * P:mt * P + mh],
                            rhs=qk_sb[0:64, ob, c0:c0 + cw],
                        )
                        nc.tensor.matmul(
                            psb[:mh, c0:c0 + cw],
                            lhsT=qk_sb[64:128, 8 + ob, mt * P:mt * P + mh],
                            rhs=qk_sb[64:128, ob, c0:c0 + cw],
                        )
                    nc.scalar.activation(
                        out=estA[:mh, mt, :], in_=psa[:mh, :N], func=Exp, scale=0.125
                    )
                    nc.scalar.activation(
                        out=estB[:mh, mt, :], in_=psb[:mh, :N], func=Exp, scale=0.125
                    )
                muls = []
                for h, est, p0 in ((2 * hp, estA, 0), (2 * hp + 1, estB, 64)):
                    use_act_otr = act_help and p0 == 0
                    pso = psA.tile([P, 640], dtf, tag="psA")
                    for (c0, cw) in NCH:
                        for mt in range(NT):
                            mh = NTS[mt]
                            nc.tensor.matmul(
                                pso[:65, c0:c0 + cw],
                                lhsT=v_sb[:mh, mt, h * 65:h * 65 + 65],
                                rhs=est[:mh, mt, c0:c0 + cw],
                                start=(mt == 0),
                                stop=(mt == NT - 1),
                            )
                    # fast psum evac; recip must read base partition 0.
                    # s1 copy rides the ACT engine (idle during PV) so the
                    # PSUM slot frees quickly without queueing behind DVE.
                    otr = n2.tile([64, N], dtb, tag="otr")
                    if use_act_otr:
                        nc.scalar.copy(out=otr[:], in_=pso[:64, :N])
                    else:
                        nc.vector.tensor_copy(out=otr[:], in_=pso[:64, :N])
                    s1 = n1.tile([1, N], dtf, tag="s1")
                    if act_help:
                        nc.scalar.copy(out=s1[0:1, :], in_=pso[64:65, :N])
                    else:
                        nc.vector.tensor_copy(out=s1[0:1, :], in_=pso[64:65, :N])
                    rec = n1.tile([1, N], dtf, tag="rec")
                    nc.vector.reciprocal_approx_fast(out=rec[0:1, :], in_=s1[0:1, :])
                    recb = n2.tile([64, N], dtf, tag="recb")
                    nc.gpsimd.partition_broadcast(recb[:], rec[0:1, :])

                    def mk(p0=p0, ob=ob, otr=otr, recb=recb):
                        nc.vector.tensor_mul(
                            out=ot_sb[p0:p0 + 64, ob, :], in0=otr[:], in1=recb[:]
                        )

                    muls.append(mk)
                return muls

            def emit_proj_seg(ot_sb, b, nt, oc, ps, seg, evac_act=False):
                nh = NTS[nt]
                for ct in range(seg * 4, seg * 4 + 4):
                    nc.tensor.matmul(
                        ps[:nh],
                        lhsT=ot_sb[:, ct, nt * P:nt * P + nh],
                        rhs=wp_sb[:, ct, oc * 512:(oc + 1) * 512],
                        start=(ct == 0),
                        stop=(ct == CT - 1 and not evac_act),
                    )
                if seg == 1:
                    outt = outpool.tile([P, 512], dtb, tag="out")
                    if evac_act:
                        # fold bias in as a K=1 ones-row matmul, evac on the
                        # tail-idle ACT engine (DVE is busy with normalize)
                        nc.tensor.matmul(
                            ps[:nh],
                            lhsT=ones1[0:1, :nh],
                            rhs=bpb_sb[0:1, oc * 512:(oc + 1) * 512],
                            start=False,
                            stop=True,
                        )
                        nc.scalar.copy(out=outt[:nh], in_=ps[:nh])
                    else:
                        nc.vector.tensor_add(
                            out=outt[:nh],
                            in0=ps[:nh],
                            in1=bpb_sb[:nh, oc * 512:(oc + 1) * 512],
                        )
                    eng = nc.sync if (nt + oc) % 2 == 0 else nc.gpsimd
                    eng.dma_start(
                        out=y[b, nt * P:nt * P + nh, oc * 512:(oc + 1) * 512],
                        in_=outt[:nh],
                    )

            def emit_proj_chunk(ot_sb, b, nt, oc, wide=False, evac_act=False):
                if wide:
                    pw = psA.tile([P, 640], dtf, tag="psA")
                    ps = pw[:, :512]
                else:
                    ps = psB.tile([P, 512], dtf, tag="psB")
                emit_proj_seg(ot_sb, b, nt, oc, ps, 0, evac_act)
                emit_proj_seg(ot_sb, b, nt, oc, ps, 1, evac_act)

            # ---- phase 0: input DMAs in first-needed order ----
            x0 = load_x(0)
            for (g0, g1) in [(0, 256), (256, 512), (512, 1024), (1024, 1536),
                             (1536, 2048)]:
                for ct in range(CT):
                    eng = nc.gpsimd if ct % 2 == 0 else nc.sync
                    eng.dma_start(
                        out=wq_sb[:, ct, g0:g1],
                        in_=wq[ct * P:(ct + 1) * P, g0:g1],
                    )
                if g0 == 0:
                    nc.sync.dma_start(out=bqk_sb[:], in_=bqk[:])
            for ct in range(CT):
                nc.sync.dma_start(
                    out=wq_sb[:, ct, 2 * C:], in_=wq[ct * P:(ct + 1) * P, 2 * C:]
                )
            nc.sync.dma_start(
                out=bvb_sb[:], in_=bass.AP(tensor=bv, offset=0, ap=[[0, P], [1, C]])
            )
            for ct in range(CT):
                nc.sync.dma_start(out=wp_sb[:, ct], in_=wp[ct * P:(ct + 1) * P, :])
            nc.sync.dma_start(
                out=bpb_sb[:], in_=bass.AP(tensor=bpr, offset=0, ap=[[0, P], [1, C]])
            )

            # ---- phase 1: QKV(b0) + V(b0), dense ----
            qk0 = qkpool.tile([P, 16, N], dtb, tag="qk")
            for ot in range(16):
                emit_qk_tile(x0, qk0, ot, wide=True)
            v0 = alloc_v(0)
            for nt in range(NT):
                for oc in range(2):
                    emit_v_chunk(x0, v0, nt, oc)

            # ---- phase 2: attention(b0) with QKV(b1)+V(b1) matmuls as PE
            # fillers between head pairs (keeps the PE dense and HAM warm) ----
            x1 = load_x(1)
            qk1 = qkpool.tile([P, 16, N], dtb, tag="qk")
            v1 = alloc_v(1)
            ot0 = opool.tile([P, CT, N], dtb, tag="ot")

            fillers = [
                lambda ot=ot: emit_qk_tile(x1, qk1, ot, wide=False)
                for ot in range(16)
            ]
            fillers += [
                lambda nt=nt: emit_v_chunk(x1, v1, nt, 0) for nt in range(NT)
            ]
            per = [3, 3, 3, 3, 3, 2, 2, 2]
            fi = 0
            for hp in range(H // 2):
                muls = emit_pair(qk0, v0, ot0, hp)
                for _ in range(per[hp]):
                    fillers[fi]()
                    fi += 1
                for m in muls:
                    m()

            # ---- phase 3: attention(b1) with proj(b0) fillers ----
            ot1 = opool.tile([P, CT, N], dtb, tag="ot")
            fillers = [
                lambda nt=nt: emit_v_chunk(x1, v1, nt, 1) for nt in range(NT)
            ]
            fillers += [
                lambda nt=nt, oc=oc: emit_proj_chunk(ot0, 0, nt, oc)
                for nt in range(NT)
                for oc in range(2)
            ]
            per = [3, 3, 2, 1, 1, 1, 2, 2]
            fi = 0
            for hp in range(H // 2):
                muls = emit_pair(qk1, v1, ot1, hp, act_help=False)
                for _ in range(per[hp]):
                    fillers[fi]()
                    fi += 1
                for m in muls:
                    m()

            # ---- phase 4: proj(b1) tail ----
            for i, (nt, oc) in enumerate(
                [(nt, oc) for nt in range(NT) for oc in range(2)]
            ):
                emit_proj_chunk(ot1, 1, nt, oc, wide=(i % 2 == 0))
    nc.compile()
    return nc


def kernel(x, w_qkv, b_qkv, w_proj, b_proj):
    global LAST_RESULT
    _ensure_ntff_hook()
    from concourse.bass_utils import run_bass_kernel_spmd

    bf16 = ml_dtypes.bfloat16
    x = np.asarray(x, dtype=np.float32)
    w_qkv = np.asarray(w_qkv, dtype=np.float32)
    b_qkv = np.asarray(b_qkv, dtype=np.float32)
    w_proj = np.asarray(w_proj, dtype=np.float32)
    b_proj = np.asarray(b_proj, dtype=np.float32)

    xT = np.ascontiguousarray(np.transpose(x, (0, 2, 1))).astype(bf16)  # [B, C, N]
    wqkvT = np.ascontiguousarray(w_qkv.T).astype(bf16)  # [C, 3C]
    wprojT = np.ascontiguousarray(w_proj.T).astype(bf16)  # [C, C]
    bqk = np.ascontiguousarray(b_qkv[:2 * C].reshape(16, P).T).astype(np.float32)
    bv = np.ascontiguousarray(b_qkv[2 * C:]).astype(bf16)
    bpr = np.ascontiguousarray(b_proj).astype(bf16)

    in_maps = []
    for i in range(NCORES):
        in_maps.append(
            {
                "xt": np.ascontiguousarray(xT[i * BPC:(i + 1) * BPC]),
                "wqkvT": wqkvT,
                "wprojT": wprojT,
                "bqk": bqk,
                "bv": bv,
                "bproj": bpr,
            }
        )

    if "nc" not in _CACHE:
        _CACHE["nc"] = _build_nc()
    nc = _CACHE["nc"]

    res = run_bass_kernel_spmd(nc, in_maps, core_ids=list(range(NCORES)))
    LAST_RESULT = res
    out = np.concatenate([r["y"] for r in res.results], axis=0)
    return np.ascontiguousarray(out.astype(np.float32))


if __name__ == "__main__":
    rng = np.random.default_rng(0)
    x = rng.standard_normal((B, N, C), dtype=np.float32)
    w_qkv = rng.standard_normal((3 * C, C), dtype=np.float32) * C ** -0.5
    b_qkv = rng.standard_normal(3 * C).astype(np.float32) * 0.02
    w_proj = rng.standard_normal((C, C), dtype=np.float32) * C ** -0.5
    b_proj = rng.standard_normal(C).astype(np.float32) * 0.02
    out = kernel(x=x, w_qkv=w_qkv, b_qkv=b_qkv, w_proj=w_proj, b_proj=b_proj)
    print(out.shape, out.dtype)

